# revision 9
# baseline (speedup 1.0000x reference)
"""Adaptive thresholding (11x11 box mean, BORDER_REPLICATE, THRESH_BINARY_INV)
on 8 TRN2 NeuronCores, data-parallel over the batch dim (16 images/core).

v4 design:
  - Host pre-bakes, per image, a [128, 4x533] fp16 plane holding y = fp16(x)/4
    with an 11-col zero head and 5-col replicate margins per segment. The /4
    scaling halves the fp16 rounding noise of the scan output. No xlo plane:
    fp16(x) quantization keeps rel-err ~1.4e-2 < 2e-2.
  - Custom DVE op ADAPT_WSCAN: out = inclusive_scan_add(Src0 - Src1), a
    single-ALU-stage recurrence at 1 elem/cycle (2x the stock scan, which
    pays a feedback bubble). One scan per image over the flat [128, 2132]
    view; zero heads drain state between segments.
  - PE per image: 12 matmuls into one [128, 4, 512] f32 PSUM tile:
      idn   (-121*I)^T @ y_seg        x4  (x-dep only: overlaps the scan)
      band  bm_{top,mid,mid,bot} @ W11 x4
      halo  bhc[10,:] @ hp[:,bank]    x4  (uniform weights, one LDW group)
    where hp [10, 4, 512] packs the cross-segment halo rows (next-seg rows
    0-4 / prev-seg rows 123-127) via 2 gpsimd SBUF->SBUF DMAs per image.
  - ONE merged Scalar activation per image: Sign(PSUM - 60.5) over 4 banks
    -> fp8e4m3 {-1,0,+1}, DMA'd out at 1 byte/px.
  - x-in / out DMA issues on sync (SP); halo DMAs on gpsimd; consts as one
    merged DMA on scalar. First/last image: chunked DMA + per-segment scans
    and matmul order to shorten pipeline fill/drain.
Host: out = (sign >= 0) * 255  (inclusive compare matches x <= mean-2).
"""
import sys
sys.path.insert(0, '/opt/trn_rl_repo')
import numpy as np
import concourse.bass as bass
import concourse.tile as tile
from concourse import bacc, mybir
from concourse.bass_utils import run_bass_kernel_spmd
from concourse import dve_ops as _dops
from concourse.dve_spec import Spec, Src0, Src1, scan, AluOp, lower
from concourse.dve_spec import _has_src1 as _hs1
from concourse.dve_uop import DveOpSpec

F32 = mybir.dt.float32
F16 = mybir.dt.float16
F8 = mybir.dt.float8e4

N_CORES = 8
BATCH, H, W = 128, 512, 512
IMGS_PER_CORE = BATCH // N_CORES      # 16
BLK = 128
NBLK = H // BLK                       # 4
K = 11
PAD = K // 2                          # 5
ZH = K                                # zero head width
WT = ZH + PAD + W + PAD               # 533 segment width
X0 = ZH + PAD                         # x offset within segment (16)
FLAT = NBLK * WT                      # 2132
SCLEN = FLAT - ZH                     # 2121 scan steps
SPAD = 3 * WT                         # 1599: strided-3-seg view length
ROWS = IMGS_PER_CORE * BLK            # 2048 partition-rows per core
CN = ("bm_top", "bm_mid", "bm_bot", "bhc", "idn")


def _register_wscan():
    name = "ADAPT_WSCAN"
    if name in _dops._SUB_OPCODE_FOR_NAME:
        return next(o for o in _dops.OPS if o.name == name)
    spec = Spec(
        body=scan(AluOp.ADD, Src0 - Src1),
        reference=lambda in0, in1, s0, s1, imm2: np.cumsum(
            in0.astype(np.float32) - in1.astype(np.float32), axis=-1),
    )
    row = _dops._CUSTOM_DVE_ROW_BASE + len(_dops.OPS)
    _dops._SUB_OPCODE_FOR_NAME[name] = row
    shas = {}
    for ver in ("v3", "v4"):
        tmp = DveOpSpec(name=name, opcode=row, uops=lower(spec, ver=ver),
                        rd1_en=_hs1(spec))
        shas[ver] = tmp.sha(ver)
    op = _dops.DveOp(name, spec, subdim=False, uops_sha=shas)
    _dops.OPS.append(op)
    _dops.CUSTOM_DVE_SPECS[name] = spec
    return op


def _band_matrices(dtype=np.float16):
    r = np.arange(BLK)
    bm_mid = (np.abs(r[:, None] - r[None, :]) <= PAD).astype(dtype)
    bm_top = bm_mid.copy()
    for rr in range(PAD):
        bm_top[0, rr] += dtype(PAD - rr)
    bm_bot = bm_mid.copy()
    for rr in range(BLK - PAD, BLK):
        bm_bot[BLK - 1, rr] += dtype(rr - (BLK - PAD - 1))
    bhp = np.zeros((BLK, BLK), dtype=dtype)
    for p in range(BLK - PAD, BLK):
        bhp[p, 0:p - (BLK - PAD) + 1] = 1.0
    bhn = np.zeros((BLK, BLK), dtype=dtype)
    for p in range(PAD):
        bhn[p, BLK - PAD + p:BLK] = 1.0
    # merged halo weights: rows 0-4 = next-seg (bhn rows 0-4), rows 32-36 =
    # prev-seg (bhp rows 123-127); rest zero (engine APs must start at a
    # partition multiple of 32, hence the 32 offset for the prev block)
    bhc = np.zeros((BLK, BLK), dtype=dtype)
    bhc[0:PAD, :] = bhn[0:PAD, :]
    bhc[32:32 + PAD, :] = bhp[BLK - PAD:BLK, :]
    idn = (-121.0 * np.eye(BLK)).astype(dtype)
    return {"bm_top": bm_top, "bm_mid": bm_mid, "bm_bot": bm_bot,
            "bhc": bhc, "idn": idn}


def _build():
    wop = _register_wscan()
    nc = bacc.Bacc(None, target_bir_lowering=False, debug=False)
    x_d = nc.declare_dram_parameter("x", [ROWS, FLAT], F16, isOutput=False)
    c_d = nc.declare_dram_parameter("consts", [BLK, len(CN) * BLK], F16,
                                    isOutput=False)
    out_d = nc.declare_dram_parameter("out", [ROWS, NBLK * W], F8, isOutput=True)

    with tile.TileContext(nc) as tc:
        with (
            tc.tile_pool(name="cpool", bufs=1) as cpool,
            tc.tile_pool(name="xin", bufs=5) as x_pool,
            tc.tile_pool(name="scr", bufs=5) as s_pool,
            tc.tile_pool(name="halo", bufs=3) as h_pool,
            tc.tile_pool(name="outp", bufs=3) as o_pool,
            tc.tile_pool(name="psum", bufs=2, space=bass.MemorySpace.PSUM) as ps_pool,
        ):
            cbig = cpool.tile([BLK, len(CN) * BLK], F16, tag="consts")
            nc.scalar.dma_start(cbig[:], c_d[:])
            ct = {nm: cbig[:, j * BLK:(j + 1) * BLK] for j, nm in enumerate(CN)}
            bias_t = cpool.tile([BLK, 1], F32, tag="bias")
            nc.vector.memset(bias_t[:], -242.0 / 4.0)

            imgs = {}
            EDGE = (0, IMGS_PER_CORE - 1)

            def segof(pos):
                return pos * WT + (K - 1)

            def front_img(i):
                ximg = x_pool.tile([BLK, NBLK, WT], F16, tag="ximg")
                xrow = x_d[i * BLK:(i + 1) * BLK, :].rearrange(
                    "q (p c) -> q p c", p=NBLK)
                # s padded to 3*WT+W past segof(1) so the strided halo views
                # stay in-bounds; scan writes only [0, SCLEN)
                s = s_pool.tile([BLK, segof(1) + SPAD], F16, tag="scr")
                flat = ximg[:].rearrange("q p c -> q (p c)")
                if i in EDGE:
                    for pos in range(NBLK):
                        nc.sync.dma_start(ximg[:, pos, :], xrow[:, pos, :])
                    for pos in range(NBLK):
                        o0 = pos * WT
                        nc.vector._custom_dve(
                            wop, out=s[:, o0:o0 + WT - ZH],
                            in0=flat[:, o0 + ZH:o0 + WT],
                            in1=flat[:, o0:o0 + WT - ZH])
                else:
                    nc.sync.dma_start(ximg[:], xrow[:])
                    nc.vector._custom_dve(
                        wop, out=s[:, 0:SCLEN], in0=flat[:, ZH:FLAT],
                        in1=flat[:, 0:SCLEN])
                # packed halo tile: [0:5, b] = next-seg rows 0-4,
                # [32:37, b] = prev-seg rows 123-127; rows 5-31 and the
                # unwritten edge pages are zeroed once per pool buffer
                hp = h_pool.tile([32 + PAD, NBLK, W], F16, tag="halo")
                if i < 3:
                    nc.gpsimd.memset(hp[0:32, :, :], 0.0)
                    nc.gpsimd.memset(hp[32:32 + PAD, 0, :], 0.0)
                nxt = s[0:PAD, segof(1):segof(1) + SPAD].rearrange(
                    "q (a c) -> q a c", a=3)[:, :, 0:W]
                nc.gpsimd.dma_start(hp[0:PAD, 0:NBLK - 1, :], nxt)
                prv = s[BLK - PAD:BLK, segof(0):segof(0) + SPAD].rearrange(
                    "q (a c) -> q a c", a=3)[:, :, 0:W]
                nc.gpsimd.dma_start(hp[32:32 + PAD, 1:NBLK, :], prv)
                imgs[i] = (ximg, s, hp)

            def back_img(i):
                ximg, s, hp = imgs.pop(i)
                ps = ps_pool.tile([BLK, NBLK, W], F32, tag="ps", name=f"ps_{i}")
                flat = ximg[:].rearrange("q p c -> q (p c)")

                bmn = ["bm_top", "bm_mid", "bm_mid", "bm_bot"]
                sseg = [s[:, segof(p):segof(p) + W] for p in range(NBLK)]
                xseg = [flat[:, p * WT + X0:p * WT + X0 + W] for p in range(NBLK)]
                for pos in range(NBLK):
                    nc.tensor.matmul(ps[:, pos, :], ct["idn"], xseg[pos],
                                     start=True, stop=False)
                for pos in range(NBLK):
                    nc.tensor.matmul(ps[:, pos, :], ct[bmn[pos]], sseg[pos],
                                     start=False, stop=False)
                for pos in range(NBLK):
                    nc.tensor.matmul(ps[:, pos, :], ct["bhc"][0:32 + PAD, :],
                                     hp[:, pos, :], start=False, stop=True)
                oimg = o_pool.tile([BLK, NBLK, W], F8, tag="oimg")
                orow = out_d[i * BLK:(i + 1) * BLK, :].rearrange(
                    "q (p c) -> q p c", p=NBLK)
                if i == IMGS_PER_CORE - 1:
                    for pos in range(NBLK):
                        nc.scalar.activation(
                            oimg[:, pos, :], ps[:, pos, :],
                            mybir.ActivationFunctionType.Sign,
                            bias=bias_t[:], scale=1.0)
                        nc.sync.dma_start(orow[:, pos, :], oimg[:, pos, :])
                else:
                    nc.scalar.activation(
                        oimg[:], ps[:], mybir.ActivationFunctionType.Sign,
                        bias=bias_t[:], scale=1.0)
                    nc.sync.dma_start(orow[:], oimg[:])

            front_img(0)
            front_img(1)
            front_img(2)
            for i in range(IMGS_PER_CORE):
                back_img(i)
                if i + 3 < IMGS_PER_CORE:
                    front_img(i + 3)
    nc.compile()
    return nc


_NC_CACHE = None


def _make_in_maps(x: np.ndarray) -> list:
    x = np.asarray(x, dtype=np.float32)
    y = (x.reshape(BATCH, H, W).astype(np.float16) / np.float16(4.0))
    yq = y.reshape(BATCH, NBLK, BLK, W).transpose(0, 2, 1, 3)
    plane = np.zeros((BATCH, BLK, NBLK, WT), dtype=np.float16)
    plane[..., X0:X0 + W] = yq
    plane[..., ZH:X0] = yq[..., 0:1]
    plane[..., X0 + W:WT] = yq[..., W - 1:W]
    cm = _band_matrices()
    cbig = np.concatenate([cm[nm] for nm in CN], axis=1)
    in_maps = []
    for c in range(N_CORES):
        shard = plane[c * IMGS_PER_CORE:(c + 1) * IMGS_PER_CORE].reshape(
            ROWS, FLAT)
        in_maps.append({"x": np.ascontiguousarray(shard),
                        "consts": np.ascontiguousarray(cbig)})
    return in_maps


def kernel(x: np.ndarray) -> np.ndarray:
    global _NC_CACHE
    if _NC_CACHE is None:
        _NC_CACHE = _build()
    nc = _NC_CACHE
    in_maps = _make_in_maps(x)
    res = run_bass_kernel_spmd(nc, in_maps, core_ids=list(range(N_CORES)))
    out = np.empty((BATCH, H, W), dtype=np.float32)
    for c in range(N_CORES):
        sgn = np.asarray(res.results[c]["out"]).view(np.uint8)
        o = (sgn < 0x80).astype(np.float32) * np.float32(255.0)
        out[c * IMGS_PER_CORE:(c + 1) * IMGS_PER_CORE] = \
            o.reshape(IMGS_PER_CORE, BLK, NBLK, W).transpose(0, 2, 1, 3).reshape(
                IMGS_PER_CORE, H, W)
    return out.reshape(BATCH, H, W, 1)
